# revision 51
# baseline (speedup 1.0000x reference)
"""Bass/Trainium2 kernel for nn_Attn (32,4096,512 attention pooling).

  energy = tanh(x @ W.T); ae = energy @ v; w = softmax(ae, axis=T)
  out[b] = sum_t w[b,t] * x[b,t,:]

Strategy (8 NeuronCores, data-parallel over B, 4 batches/core):
  - host casts x to bf16 (halves HBM traffic; PE fp32 matmul is 4x slow
    anyway so bf16 is also the fast matmul path)
  - x is loaded twice per batch: natural layout [t,h] (for the weighted
    sum) and DMA-xbar-transposed [h,t] (for the energy matmul contraction
    over h)
  - energy^T computed g-partitioned so the v-dot is a PE matmul
  - softmax skips the max subtraction: |ae| <= sum|v| ~ 18, exp stays in
    fp32 range, so exp/sum is computed unnormalized and the output is
    scaled by 1/S at the end
  - w (exp(ae), [1,T] on one partition) is re-laid-out to [128,T/128] via
    a tiny DRAM round-trip + DMA transpose for the weighted-sum matmul
"""

import numpy as np
import ml_dtypes
from contextlib import ExitStack

import bass_rust
import concourse.bass as bass
import concourse.mybir as mybir
import concourse.tile as tile
from concourse import bass_isa
from concourse.bass_utils import run_bass_kernel_spmd

# ---------------------------------------------------------------------------
# Workaround: this container's walrus accepts only ONE sem-wait per
# instruction. Tile's sem-assignment can attach several. Split the extras
# onto same-engine NoOps inserted immediately before the instruction —
# the engine queue executes them in order, so all waits still complete
# before the instruction dispatches.


def _split_excess_waits(nc, max_waits=1):
    n_split = 0
    for fn in nc.m.functions:
        for blk in fn.blocks:
            new = []
            changed = False
            for inst in blk.instructions:
                si = inst.sync_info
                waits = list(si.on_wait) if si is not None else []
                if len(waits) > max_waits:
                    for w in waits[:-max_waits]:
                        nop = mybir.InstNoOp(
                            name=nc.get_next_instruction_name(),
                            engine=inst.engine,
                            ins=[],
                            outs=[],
                            sync_info=bass_rust.SyncInfo(
                                on_wait=[w], on_update=[]
                            ),
                        )
                        new.append(nop)
                        n_split += 1
                    inst.sync_info = bass_rust.SyncInfo(
                        on_wait=waits[-max_waits:], on_update=list(si.on_update)
                    )
                    changed = True
                new.append(inst)
            if changed:
                blk.instructions = new
    return n_split
# ---------------------------------------------------------------------------

B, T, H = 32, 4096, 512
N_CORES = 8
B_LOC = B // N_CORES          # batches per core
PC = 128                      # partitions
HC = H // PC                  # 4 h-chunks
GC = H // PC                  # 4 g-chunks
TBLK = 512                    # tokens per pipeline block
NBLK = T // TBLK              # 8 blocks per batch
NT = T // PC                  # 32 token subtiles per batch

BF16 = mybir.dt.bfloat16
F16 = mybir.dt.float16
F32 = mybir.dt.float32
AF = mybir.ActivationFunctionType


USE_TILEPOS = True


def _tp(c):
    return (0, 32 * c) if USE_TILEPOS else None


def _build_program(reps=1, split_waits=True):
    nc = bass.Bass()
    x_d = nc.declare_dram_parameter("x", [B_LOC, T, H], BF16, isOutput=False)
    wt_d = nc.declare_dram_parameter("wt", [H, H], BF16, isOutput=False)
    v_d = nc.declare_dram_parameter("v", [H], BF16, isOutput=False)
    out_d = nc.declare_dram_parameter("out", [B_LOC, H], F32, isOutput=True)

    with tile.TileContext(nc) as tc, ExitStack() as ctx:
        singles = ctx.enter_context(tc.tile_pool(name="singles", bufs=1))
        xnatp = ctx.enter_context(tc.tile_pool(name="xnat", bufs=2))
        xtp = ctx.enter_context(tc.tile_pool(name="xt", bufs=2))
        tanhp = ctx.enter_context(tc.tile_pool(name="tanh", bufs=4))
        wflatp = ctx.enter_context(tc.tile_pool(name="wflat", bufs=2))
        smallp = ctx.enter_context(tc.tile_pool(name="small", bufs=8))
        wcolsp = ctx.enter_context(tc.tile_pool(name="wcols", bufs=4))
        osbp = ctx.enter_context(tc.tile_pool(name="osb", bufs=2))
        dramp = ctx.enter_context(tc.tile_pool(name="dram", bufs=2, space="DRAM"))
        pep = ctx.enter_context(tc.tile_pool(name="pe", bufs=3, space="PSUM"))
        pvp = ctx.enter_context(tc.tile_pool(name="pv", bufs=2, space="PSUM"))
        pop = ctx.enter_context(tc.tile_pool(name="po", bufs=1, space="PSUM"))

        # constants: WT as [p, hc, g] (partition = h within chunk), v as [p, gc]
        wt_sb = singles.tile([PC, HC, H], BF16)
        nc.sync.dma_start(out=wt_sb, in_=wt_d.rearrange("(hc p) g -> p hc g", p=PC))
        v_sb = singles.tile([PC, GC], BF16)
        nc.sync.dma_start(out=v_sb, in_=v_d.rearrange("(gc p) -> p gc", p=PC))
        ones_sb = singles.tile([PC, 1], BF16)
        nc.vector.memset(ones_sb, 1.0)
        v32_sb = singles.tile([PC, GC], F32)
        nc.vector.tensor_copy(v32_sb, v_sb)
        ones16_sb = singles.tile([PC, 1], F16)
        nc.vector.memset(ones16_sb, 1.0)

        def body():
            for b in range(B_LOC):
                _one_batch(nc, tc, b, x_d, out_d, wt_sb, v_sb, ones_sb, pools, v32_sb, ones16_sb)

        vaccp = ctx.enter_context(tc.tile_pool(name="vacc", bufs=4))
        vredp = ctx.enter_context(tc.tile_pool(name="vred", bufs=2))
        pools = dict(
            xnatp=xnatp, xtp=xtp, tanhp=tanhp, wflatp=wflatp, smallp=smallp,
            wcolsp=wcolsp, osbp=osbp, dramp=dramp, pep=pep, pvp=pvp, pop=pop,
            vaccp=vaccp, vredp=vredp,
        )
        if reps == 1:
            body()
        else:
            with tc.For_i(0, reps, 1):
                body()

    if split_waits:
        _split_excess_waits(nc)
    return nc


def _one_batch(nc, tc, b, x_d, out_d, wt_sb, v_sb, ones_sb, pools, v32_sb, ones16_sb):
    xnatp = pools["xnatp"]; xtp = pools["xtp"]; tanhp = pools["tanhp"]
    wflatp = pools["wflatp"]; smallp = pools["smallp"]; wcolsp = pools["wcolsp"]
    osbp = pools["osbp"]; dramp = pools["dramp"]
    pep = pools["pep"]; pvp = pools["pvp"]; pop = pools["pop"]

    # natural layout: [p, nt, h], token = nt*128 + p (ACT HWDGE queue,
    # separate FIFO from the SP queue carrying the xbar transposes)
    xn = xnatp.tile([PC, NT, H], BF16)
    nc.scalar.dma_start(out=xn, in_=x_d[b].rearrange("(n p) h -> p n h", p=PC))
    # transposed layout: [p, hc, t], h = hc*128 + p  (HWDGE xbar)
    xt = xtp.tile([PC, HC, T], BF16)
    for hc in range(HC):
        nc.sync.dma_start_transpose(
            out=xt[:, hc, :], in_=x_d[b, :, hc * PC : (hc + 1) * PC]
        )

    # attn energies ae[t] (pre-exp), fp16 for the 2-byte xbar transpose
    wf = wflatp.tile([1, T], F16)
    # v-dot off PE: DVE does the per-partition v-scale + chunk accumulate,
    # GPSIMD does the partition reduction (Pool is otherwise idle)
    vaccp = pools["vaccp"]; vredp = pools["vredp"]
    for blk in range(NBLK):
        t0 = blk * TBLK
        tE_g = []
        for gc in range(GC):
            pe_t = pep.tile([PC, TBLK], F32)
            for hc in range(HC):
                nc.tensor.matmul(
                    pe_t,
                    lhsT=wt_sb[:, hc, gc * PC : (gc + 1) * PC],
                    rhs=xt[:, hc, t0 : t0 + TBLK],
                    start=(hc == 0),
                    stop=(hc == HC - 1),
                )
            tE = tanhp.tile([PC, TBLK], BF16)
            nc.scalar.activation(out=tE, in_=pe_t, func=AF.Tanh)
            tE_g.append(tE)
        accs = []
        for _i in range(GC):
            acc_t = vaccp.tile([PC, TBLK], F16, tag="vacc", name=f"acc_{blk}_{_i}")
            accs.append(acc_t)
        nc.vector.tensor_scalar(
            out=accs[0], in0=tE_g[0], scalar1=v32_sb[:, 0:1], scalar2=None,
            op0=mybir.AluOpType.mult,
        )
        for gc in range(1, GC):
            nc.vector.scalar_tensor_tensor(
                out=accs[gc], in0=tE_g[gc], scalar=v32_sb[:, gc : gc + 1],
                in1=accs[gc - 1], op0=mybir.AluOpType.mult,
                op1=mybir.AluOpType.add,
            )
        pv_t = pvp.tile([1, TBLK], F32)
        nc.tensor.matmul(pv_t, lhsT=ones16_sb, rhs=accs[GC - 1], start=True, stop=True)
        nc.vector.tensor_copy(wf[:, t0 : t0 + TBLK], pv_t)

    # relayout ae: [1,T] (one partition) -> [128, NT] via DRAM + xbar
    wd = dramp.tile([NT, PC], F16)
    nc.sync.dma_start(out=wd, in_=wf)
    wcr = wcolsp.tile([PC, NT], F16)
    nc.sync.dma_start_transpose(out=wcr, in_=wd)
    # exp on the [128, NT] layout (all 128 lanes), unnormalized weights
    wc = wcolsp.tile([PC, NT], BF16)
    nc.scalar.activation(out=wc, in_=wcr, func=AF.Exp)

    # S = sum of exp weights: ones-matmul -> [1, NT] -> reduce
    ps_t = pvp.tile([1, NT], F32, tag="pv_t")
    nc.tensor.matmul(ps_t, lhsT=ones_sb, rhs=wc, start=True, stop=True)
    S = smallp.tile([1, 1], F32)
    rS = smallp.tile([1, 1], F32)
    nc.vector.tensor_reduce(
        out=S, in_=ps_t, axis=mybir.AxisListType.X, op=mybir.AluOpType.add
    )
    nc.vector.reciprocal(rS, S)

    # weighted sum: 32 M=1 matmuls, 4-way col-tiled to strips 0/32/64/96
    po_t = pop.tile([PC, H], F32)
    for j in range(NT):
        c = j % 4
        nc.tensor.matmul(
            po_t[32 * c : 32 * c + 1, :],
            lhsT=wc[:, j : j + 1],
            rhs=xn[:, j, :],
            start=(j < 4),
            stop=(j >= NT - 4),
            tile_position=_tp(c),
        )
    # combine strips (TT may read only one PSUM operand) + normalize by 1/S
    c0 = osbp.tile([1, H], F32)
    nc.vector.tensor_copy(c0, po_t[0:1, :])
    s1 = osbp.tile([1, H], F32)
    nc.vector.tensor_add(s1, c0, po_t[32:33, :])
    s2 = osbp.tile([1, H], F32)
    nc.vector.tensor_add(s2, s1, po_t[64:65, :])
    s3 = osbp.tile([1, H], F32)
    nc.vector.tensor_add(s3, s2, po_t[96:97, :])
    ob = osbp.tile([1, H], F32)
    nc.vector.tensor_scalar_mul(ob, s3, rS)
    nc.sync.dma_start(out=out_d[b : b + 1, :], in_=ob)


_PROGRAM = None


def _get_program():
    global _PROGRAM
    if _PROGRAM is None:
        _PROGRAM = _build_program()
    return _PROGRAM


def run(inputs, trace=False, trace_kwargs=None):
    x = np.asarray(inputs["encoder_outputs"])
    W = np.asarray(inputs["W"])
    v = np.asarray(inputs["v"])
    assert x.shape == (B, T, H)

    xb = x.astype(ml_dtypes.bfloat16)
    wtb = np.ascontiguousarray(W.T).astype(ml_dtypes.bfloat16)
    vb = v.astype(ml_dtypes.bfloat16)

    in_maps = [
        {"x": xb[c * B_LOC : (c + 1) * B_LOC], "wt": wtb, "v": vb}
        for c in range(N_CORES)
    ]
    nc = _get_program()
    res = run_bass_kernel_spmd(
        nc,
        in_maps,
        list(range(N_CORES)),
        trace=trace,
        **(trace_kwargs or {}),
    )
    out = np.concatenate([res.results[c]["out"] for c in range(N_CORES)], axis=0)
    return out.astype(np.float32), res


def kernel(**inputs):
    out, _ = run(inputs)
    return out
